# revision 3
# baseline (speedup 1.0000x reference)
"""Relational GCN layer (gnn_message_passing) on 8 TRN2 NeuronCores.

Math (per reference):
    agg[r] = segment_sum(vals[r][:,None] * inp[src[r]], dst[r], N)   # [N, F]
    out    = sum_r agg[r] @ W[r]                                     # [N, F]

Strategy: shard OUTPUT NODES (dst) across the 8 cores (6250 nodes each) —
no collective needed (collectives on-chip are ~5x slower than HBM).
Each core handles the ~100k edges (across all 8 relations) whose dst lands
in its slice:

  1. dma_gather the edges' src rows from the full `inp` (f32, 512B rows).
  2. Cast gathered messages to bf16 (DVE, 2x mode).
  3. Aggregation via "segment matmul": for each (dst-tile of 128 nodes,
     relation) group, S^T[edge_lane, node_col] = val_e * (dst_e == col).
     aggT_tile = msgs_chunk.T @ S_chunk accumulated in PSUM -> [f_in, n]
     directly transposed for the weight GEMM (no explicit transposes).
     S is built host-side (it is a data-layout transform of vals/dst,
     no float arithmetic) and DMA'd in bf16.
  4. out_tile[n, f_out] = sum_r aggT(t,r).T @ W[r], accumulated in PSUM.

SPMD constraint: one program for all 8 cores -> chunk layout is padded to
the max across cores per (tile, relation, src-half) group. Pad lanes gather
row 0 (valid index; descriptors are cheap) and their S columns are zero, so
they contribute nothing.

src indices must fit in int16 for dma_gather -> edges are split into
src < 32768 ("lo") and src >= 32768 ("hi", gathered with a base offset).
"""

import numpy as np
import ml_dtypes

# Problem constants (hardcoded per harness contract).
N, R, E, F = 50000, 8, 100000, 128
C = 8                      # cores
NPC = N // C               # 6250 dst nodes per core
TILE = 128                 # dst nodes per tile
T = -(-NPC // TILE)        # 49 tiles per core (last has 106 nodes)
HALF = 32768               # int16 gather-index boundary
BATCH = 64                 # chunks (of 128 edges) per pipeline batch

F32 = np.float32
BF16 = ml_dtypes.bfloat16


def _build_layout(src, dst, vals):
    """Compute the shared chunk structure + per-core device arrays.

    Returns (meta, per_core) where meta describes the shared program
    structure and per_core[c] holds the input arrays for core c.
    """
    src = np.asarray(src)
    dst = np.asarray(dst)
    vals = np.asarray(vals)

    # --- partition edges into (core, tile, relation, half) groups ---------
    # edges[c][t][r] = (lo_src, lo_dstloc, lo_val, hi_src, hi_dstloc, hi_val)
    group_edges = [[[None] * R for _ in range(T)] for _ in range(C)]
    for r in range(R):
        d = dst[r]
        s = src[r]
        v = vals[r]
        order = np.argsort(d, kind="stable")
        ds = d[order]
        for c in range(C):
            a = np.searchsorted(ds, c * NPC, "left")
            b = np.searchsorted(ds, (c + 1) * NPC, "left")
            eidx = order[a:b]
            dl = ds[a:b] - c * NPC
            for t in range(T):
                ta = np.searchsorted(dl, t * TILE, "left")
                tb = np.searchsorted(dl, (t + 1) * TILE, "left")
                ge = eidx[ta:tb]
                gs = s[ge]
                lo = gs < HALF
                group_edges[c][t][r] = (
                    gs[lo], (d[ge] - (c * NPC + t * TILE))[lo], v[ge][lo],
                    gs[~lo], (d[ge] - (c * NPC + t * TILE))[~lo], v[ge][~lo],
                )

    # --- shared chunk counts (max over cores) -----------------------------
    n_lo = np.zeros((T, R), int)
    n_hi = np.zeros((T, R), int)
    for t in range(T):
        for r in range(R):
            n_lo[t, r] = max(-(-len(group_edges[c][t][r][0]) // 128) for c in range(C))
            n_hi[t, r] = max(-(-len(group_edges[c][t][r][3]) // 128) for c in range(C))

    # Groups in (tile, relation) order; skip empty groups.
    groups = []  # (t, r, nlo, nhi)
    for t in range(T):
        for r in range(R):
            if n_lo[t, r] + n_hi[t, r] > 0:
                groups.append((t, r, int(n_lo[t, r]), int(n_hi[t, r])))

    # --- greedy batches of whole groups, <= BATCH chunks each -------------
    batches = []  # list of dicts
    cur = []
    cur_sz = 0
    for g in groups:
        sz = g[2] + g[3]
        if cur and cur_sz + sz > BATCH:
            batches.append(cur)
            cur = []
            cur_sz = 0
        cur.append(g)
        cur_sz += sz
    if cur:
        batches.append(cur)

    # --- per-batch slot layout: lo chunks first, then hi ------------------
    meta_batches = []
    smat_cols = 0     # cumulative chunk slots over all batches
    lo_cols = 0       # cumulative idx columns (n_lo_batch * 8) for lo calls
    hi_cols = 0
    for bg in batches:
        blo = sum(g[2] for g in bg)
        bhi = sum(g[3] for g in bg)
        ginfo = []
        lo_off = 0
        hi_off = 0
        for (t, r, glo, ghi) in bg:
            ginfo.append(dict(t=t, r=r, nlo=glo, nhi=ghi,
                              lo_off=lo_off, hi_off=hi_off))
            lo_off += glo
            hi_off += ghi
        meta_batches.append(dict(
            groups=ginfo, n_lo=blo, n_hi=bhi,
            smat_base=smat_cols, lo_base=lo_cols, hi_base=hi_cols,
        ))
        smat_cols += blo + bhi
        lo_cols += blo * 8
        hi_cols += bhi * 8

    # first/last group index per tile (for out-PSUM start/stop + writeback)
    tile_first = {}
    tile_last = {}
    gi = 0
    for bg in meta_batches:
        for g in bg["groups"]:
            t = g["t"]
            if t not in tile_first:
                tile_first[t] = gi
            tile_last[t] = gi
            gi += 1

    meta = dict(batches=meta_batches, nchunk=smat_cols,
                lo_cols=lo_cols, hi_cols=hi_cols,
                tile_first=tile_first, tile_last=tile_last)

    # --- per-core data arrays --------------------------------------------
    per_core = []
    for c in range(C):
        gidx_lo = np.zeros((16, max(lo_cols, 8)), np.int16)
        gidx_hi = np.zeros((16, max(hi_cols, 8)), np.int16)
        smat = np.zeros((128, max(smat_cols, 1) * 128), BF16)
        for bg in meta_batches:
            for g in bg["groups"]:
                (ls, ld, lv, hs, hd, hv) = group_edges[c][g["t"]][g["r"]]
                # lo half: idx stream position = (lo_off + i//128)*128 + i%128
                # within the batch's lo call; pad idx stays 0 (gathers row 0).
                for (es, ed, ev, nch, off, base, arr, sub) in (
                    (ls, ld, lv, g["nlo"], g["lo_off"], bg["lo_base"], gidx_lo, 0),
                    (hs, hd, hv, g["nhi"], g["hi_off"], bg["hi_base"], gidx_hi, HALF),
                ):
                    if nch == 0:
                        continue
                    ne = len(es)
                    vec = np.zeros(nch * 128, np.int16)
                    vec[:ne] = (es - sub).astype(np.int16)
                    # wrap to [16, nch*8] and place at the call's column base
                    wr = vec.reshape(-1, 16).T
                    arr[:, base + off * 8: base + (off + nch) * 8] = wr
                    # S^T entries: lane i%128, chunk slot = smat_base +
                    # (lo: off + i//128 | hi: n_lo + off + i//128)
                    i = np.arange(ne)
                    slot = bg["smat_base"] + (0 if sub == 0 else bg["n_lo"]) \
                        + off + i // 128
                    col = slot * 128 + ed
                    smat[i % 128, col] = ev.astype(BF16)
        per_core.append(dict(
            gidx_lo=np.tile(gidx_lo, (8, 1)),
            gidx_hi=np.tile(gidx_hi, (8, 1)),
            smat=smat,
        ))
    return meta, per_core


def _build_program(meta):
    import concourse.bacc as bacc
    import concourse.bass as bass
    import concourse.mybir as mybir
    import concourse.tile as tile

    f32 = mybir.dt.float32
    bf16 = mybir.dt.bfloat16
    i16 = mybir.dt.int16

    nc = bacc.Bacc(None, target_bir_lowering=False)

    inp_d = nc.dram_tensor("inp", [N, F], f32, kind="ExternalInput")
    gilo_d = nc.dram_tensor("gidx_lo", [128, max(meta["lo_cols"], 8)], i16,
                            kind="ExternalInput")
    gihi_d = nc.dram_tensor("gidx_hi", [128, max(meta["hi_cols"], 8)], i16,
                            kind="ExternalInput")
    smat_d = nc.dram_tensor("smat", [128, max(meta["nchunk"], 1) * 128], bf16,
                            kind="ExternalInput")
    w_d = nc.dram_tensor("weights", [R, F, F], f32, kind="ExternalInput")
    out_d = nc.dram_tensor("out", [NPC, F], f32, kind="ExternalOutput")

    with tile.TileContext(nc) as tc:
        with (
            tc.tile_pool(name="const", bufs=1) as cpool,
            tc.tile_pool(name="gbuf", bufs=2) as gpool,
            tc.tile_pool(name="msgs", bufs=2) as mpool,
            tc.tile_pool(name="stile", bufs=2) as spool,
            tc.tile_pool(name="aggT", bufs=4) as apool,
            tc.tile_pool(name="osb", bufs=2) as opool,
            tc.tile_pool(name="psA", bufs=2, space="PSUM") as psum_a,
            tc.tile_pool(name="psO", bufs=2, space="PSUM") as psum_o,
        ):
            wtile = cpool.tile([128, R * F], f32)
            for r in range(R):
                nc.sync.dma_start(wtile[:, r * F:(r + 1) * F], w_d[r])
            gilo = cpool.tile([128, max(meta["lo_cols"], 8)], i16)
            nc.sync.dma_start(gilo[:], gilo_d[:])
            gihi = cpool.tile([128, max(meta["hi_cols"], 8)], i16)
            nc.sync.dma_start(gihi[:], gihi_d[:])

            out_ps = {}   # tile t -> psum tile
            gidx = 0
            for bg in meta["batches"]:
                blo, bhi = bg["n_lo"], bg["n_hi"]
                ntot = blo + bhi
                gbuf = gpool.tile([128, BATCH, F], f32, tag="gbuf")
                if blo:
                    nc.gpsimd.dma_gather(
                        gbuf[:, 0:blo, :], inp_d[0:HALF, :],
                        gilo[:, bg["lo_base"]: bg["lo_base"] + blo * 8],
                        blo * 128, blo * 128, F, single_packet=False,
                    )
                if bhi:
                    nc.gpsimd.dma_gather(
                        gbuf[:, blo:ntot, :], inp_d[HALF:N, :],
                        gihi[:, bg["hi_base"]: bg["hi_base"] + bhi * 8],
                        bhi * 128, bhi * 128, F, single_packet=False,
                    )
                msgs = mpool.tile([128, BATCH, F], bf16, tag="msgs")
                nc.vector.tensor_copy(msgs[:, 0:ntot, :], gbuf[:, 0:ntot, :])
                stile = spool.tile([128, BATCH, F], bf16, tag="stile")
                nc.sync.dma_start(
                    stile[:, 0:ntot, :],
                    smat_d[:, bg["smat_base"] * 128:(bg["smat_base"] + ntot) * 128],
                )

                for g in bg["groups"]:
                    t, r = g["t"], g["r"]
                    chunks = [g["lo_off"] + k for k in range(g["nlo"])] + \
                             [blo + g["hi_off"] + k for k in range(g["nhi"])]
                    agg = psum_a.tile([128, F], f32, tag="agg")
                    for ci, ch in enumerate(chunks):
                        nc.tensor.matmul(
                            agg[:], msgs[:, ch, :], stile[:, ch, :],
                            start=(ci == 0), stop=(ci == len(chunks) - 1),
                        )
                    aggT = apool.tile([128, F], f32, tag="aggT")
                    nc.vector.tensor_copy(aggT[:], agg[:])

                    if meta["tile_first"][t] == gidx:
                        out_ps[t] = psum_o.tile([128, F], f32, tag="ops",
                                                name="ops")
                    nc.tensor.matmul(
                        out_ps[t][:], aggT[:], wtile[:, r * F:(r + 1) * F],
                        start=(meta["tile_first"][t] == gidx),
                        stop=(meta["tile_last"][t] == gidx),
                        skip_group_check=True,
                    )
                    if meta["tile_last"][t] == gidx:
                        osb = opool.tile([128, F], f32, tag="osb")
                        nc.vector.tensor_copy(osb[:], out_ps[t][:])
                        rows = min(TILE, NPC - t * TILE)
                        nc.sync.dma_start(
                            out_d[t * TILE: t * TILE + rows, :], osb[0:rows, :])
                        del out_ps[t]
                    gidx += 1

    nc.compile()
    return nc


def kernel(inp, src, dst, vals, weights):
    from concourse.bass_utils import run_bass_kernel_spmd

    inp = np.asarray(inp, F32)
    weights = np.asarray(weights, F32)
    meta, per_core = _build_layout(src, dst, vals)
    nc = _build_program(meta)

    in_maps = [
        dict(inp=inp, weights=weights,
             gidx_lo=pc["gidx_lo"], gidx_hi=pc["gidx_hi"], smat=pc["smat"])
        for pc in per_core
    ]
    res = run_bass_kernel_spmd(nc, in_maps, core_ids=list(range(C)))
    out = np.concatenate([res.results[c]["out"] for c in range(C)], axis=0)
    return out.astype(F32)
